# revision 5
# baseline (speedup 1.0000x reference)
"""AnchorGenerator Bass kernel for 8 Trainium2 NeuronCores.

Generates multi-level anchor boxes: for each of 4 feature-map levels
(stride 4/8/16/32, sizes 1024^2/512^2/256^2/128^2), the output is
[9*h*w, 4] f32 rows (cx, cy, aw, ah), ordered (anchor_type, y, x).

Strategy (memory-bound, pure output generation ~200MB):
  - Shard each level's flattened rows contiguously across the 8 cores:
    core k owns rows [k*N/8, (k+1)*N/8). Since h % 128 == 0 for every
    level, a 128-row-group tile never straddles an anchor-type boundary,
    and the host gather is a plain concatenate.
  - Per core+level the shard is [G8 row-groups, 4w floats]; tiles of 128
    row-groups are built in SBUF and DMA'd out as single large
    contiguous transfers (2MB for level 0).
  - Tile content: col0 = xc (iota-generated, identical for every tile of
    a level) lives in persistent template buffers written once; cols
    1/2/3 (y center / anchor w / anchor h) are per-row-group constants
    rewritten per tile from a tiny host-precomputed table via
    per-partition-scalar ops (vector engine x2, scalar engine x1).
"""

import numpy as np

import concourse.bass as bass
import concourse.bacc as bacc
import concourse.mybir as mybir
from concourse.tile import TileContext
from concourse.bass_utils import run_bass_kernel_spmd

NCORES = 8
STRIDES = [4, 8, 16, 32]
SIZES = [(1024, 1024), (512, 512), (256, 256), (128, 128)]
S = 3  # scales
R = 3  # ratios
A = S * R

# Per-level derived constants: (h, w, stride, G8, T)
#   G8 = row-groups (of w rows) per core, T = 128-row-group tiles
LEVELS = []
for (_h, _w), _s in zip(SIZES, STRIDES):
    assert _h % 128 == 0
    _G8 = A * _h // NCORES
    LEVELS.append((_h, _w, _s, _G8, (_G8 + 127) // 128))
TTOT = sum(lv[4] for lv in LEVELS)  # total tiles per core (19)
NBUFS = [6, 4, 2, 2]  # persistent template buffers per level

_F32 = mybir.dt.float32

_BUILT = None  # cached build


def _build():
    """Build the Bass program (identical on all 8 cores; per-core data
    arrives via the small scalar-table input)."""
    nc = bacc.Bacc()

    # scal[p, 3*tile + {0,1,2}] = (y center, anchor w, anchor h) for
    # partition p of that tile.
    scal = nc.dram_tensor("scal", [128, 3 * TTOT], _F32, kind="ExternalInput")
    outs = [
        nc.dram_tensor(f"out{L}", [G8, 4 * w], _F32, kind="ExternalOutput")
        for L, (h, w, s, G8, T) in enumerate(LEVELS)
    ]

    ident = mybir.ActivationFunctionType.Identity
    mult, add = mybir.AluOpType.mult, mybir.AluOpType.add
    with TileContext(nc) as tc:
        with tc.tile_pool(name="pool", bufs=1) as pool:
            sc = pool.tile([128, 3 * TTOT], _F32, tag="sc", name="sc")
            nc.sync.dma_start(out=sc[:, :], in_=scal[:, :])

            tbase = 0
            for L, (h, w, stride, G8, T) in enumerate(LEVELS):
                # xc[p, x] = x*stride + stride//2 (exact small ints in f32)
                xc = pool.tile([128, w], _F32, tag=f"xc{L}", name=f"xc{L}")
                nc.gpsimd.iota(
                    xc[:, :],
                    pattern=[[stride, w]],
                    base=stride // 2,
                    channel_multiplier=0,
                    allow_small_or_imprecise_dtypes=True,
                )
                B = NBUFS[L]
                bufs = [
                    pool.tile([128, 4 * w], _F32, tag=f"b{L}_{b}", name=f"b{L}_{b}")
                    for b in range(B)
                ]
                for t in range(T):
                    P = min(128, G8 - t * 128)
                    buf = bufs[t % B]
                    if t < B:  # one-time x-center template fill
                        nc.vector.tensor_copy(
                            buf.rearrange("p (x c) -> p x c", c=4)[:, :, 0],
                            xc[:, :],
                        )
                    tv = buf[:P, :].rearrange("p (x c) -> p x c", c=4)
                    c0 = 3 * (tbase + t)
                    nc.vector.tensor_scalar(
                        tv[:, :, 1], xc[:P, :], 0.0, sc[:P, c0 : c0 + 1],
                        mult, add,
                    )
                    nc.vector.tensor_scalar(
                        tv[:, :, 2], xc[:P, :], 0.0, sc[:P, c0 + 1 : c0 + 2],
                        mult, add,
                    )
                    nc.scalar.activation(
                        tv[:, :, 3], xc[:P, :], ident,
                        bias=sc[:P, c0 + 2 : c0 + 3], scale=0.0,
                    )
                    nc.sync.dma_start(
                        out=outs[L][t * 128 : t * 128 + P, :], in_=buf[:P, :]
                    )
                tbase += T
    nc.finalize()
    return nc


def _tables(scales, ratios):
    """Host-precomputed per-core scalar tables [8, 128, 3*TTOT] f32."""
    scales = np.asarray(scales, dtype=np.float32)
    ratios = np.asarray(ratios, dtype=np.float32)
    tabs = np.zeros((NCORES, 128, 3 * TTOT), np.float32)
    p = np.arange(128)
    tbase = 0
    for (h, w, stride, G8, T) in LEVELS:
        base = scales * np.float32(stride)               # [S] f32
        sr = np.sqrt(ratios)                             # [R] f32
        aw = (base[:, None] * sr[None, :]).reshape(-1)   # [A] f32
        ah = (base[:, None] / sr[None, :]).reshape(-1)   # [A] f32
        for k in range(NCORES):
            for t in range(T):
                g = np.minimum(k * G8 + t * 128 + p, (k + 1) * G8 - 1)
                a = g // h
                y = g % h
                c0 = 3 * (tbase + t)
                tabs[k, :, c0] = (y * stride + stride // 2).astype(np.float32)
                tabs[k, :, c0 + 1] = aw[a]
                tabs[k, :, c0 + 2] = ah[a]
        tbase += T
    return tabs


def _get_built():
    global _BUILT
    if _BUILT is None:
        _BUILT = _build()
    return _BUILT


def _run(scales, ratios, **spmd_kwargs):
    nc = _get_built()
    tabs = _tables(scales, ratios)
    in_maps = [{"scal": tabs[k]} for k in range(NCORES)]
    res = run_bass_kernel_spmd(nc, in_maps, core_ids=list(range(NCORES)),
                               **spmd_kwargs)
    outs = []
    for L, (h, w, stride, G8, T) in enumerate(LEVELS):
        full = np.concatenate(
            [res.results[k][f"out{L}"] for k in range(NCORES)], axis=0
        )
        outs.append(full.reshape(A * h * w, 4))
    return tuple(outs), res


def kernel(scales, ratios, fs0_h, fs0_w, fs1_h, fs1_w, fs2_h, fs2_w,
           fs3_h, fs3_w):
    sizes = [(int(fs0_h), int(fs0_w)), (int(fs1_h), int(fs1_w)),
             (int(fs2_h), int(fs2_w)), (int(fs3_h), int(fs3_w))]
    assert sizes == SIZES, f"kernel compiled for {SIZES}, got {sizes}"
    outs, _ = _run(scales, ratios)
    return outs
